# revision 1
# baseline (speedup 1.0000x reference)
"""Trainium2 Bass kernel: transformer block with dilated (parity-strided,
banded, causal) attention.  See kernel design notes in the module docstring
of the original; this revision folds both LayerNorms into the surrounding
GEMMs:

    LN(x) @ W'^T  ==  rstd * (x @ W'^T  -  mu * rowsum(W'))

so the projections consume raw bf16 x plus one rank-1 matmul term
(mean row x host-precomputed weight row-sums), and the rstd scale is
applied in the PSUM epilogue.  The GEMMs therefore wait only on the mean,
not on the full variance/rsqrt chain, which removes the two LN serial
walls from the PE critical path.
"""

import numpy as np
import ml_dtypes

import concourse.bass as bass
import concourse.bacc as bacc
import concourse.mybir as mybir
import concourse.tile as tile
from concourse.bass_utils import run_bass_kernel_spmd

BF16NP = ml_dtypes.bfloat16
F32 = mybir.dt.float32
BF16 = mybir.dt.bfloat16
AF = mybir.ActivationFunctionType
OP = mybir.AluOpType

P = 128
B, L, E = 2, 2048, 768
ET = E // P            # 6 tiles over E
H, D = 12, 64
MLP = 4 * E            # 3072
MT = MLP // P          # 24
OWN = 512              # tokens owned per core
HALO = 256             # preceding-context tokens
SLAB = OWN + HALO      # 768
EPS = 1e-5
N_CORES = 8


def _fold2(apv):
    """[.., T] -> [.., 2, T//2] parity view of a stride-1 token axis."""
    return apv.rearrange("... (t two) -> ... two t", two=2)


def build_program():
    nc = bacc.Bacc("TRN2", target_bir_lowering=False, debug=False)

    xT = nc.dram_tensor("xT", [E, SLAB], F32, kind="ExternalInput").ap()
    qkv_wT = nc.dram_tensor("qkv_wT", [E, 3 * E], BF16, kind="ExternalInput").ap()
    out_wT = nc.dram_tensor("out_wT", [E, E], BF16, kind="ExternalInput").ap()
    ffn_w1T = nc.dram_tensor("ffn_w1T", [E, MLP], BF16, kind="ExternalInput").ap()
    ffn_w2T = nc.dram_tensor("ffn_w2T", [MLP, E], BF16, kind="ExternalInput").ap()
    qkv_b = nc.dram_tensor("qkv_b", [3 * E], F32, kind="ExternalInput").ap()
    out_b = nc.dram_tensor("out_b", [E], F32, kind="ExternalInput").ap()
    ffn_b1 = nc.dram_tensor("ffn_b1", [MLP], F32, kind="ExternalInput").ap()
    ffn_b2 = nc.dram_tensor("ffn_b2", [E], F32, kind="ExternalInput").ap()
    maskT = nc.dram_tensor("maskT", [2, 2, P, P], BF16, kind="ExternalInput").ap()
    yT = nc.dram_tensor("yT", [E, OWN], F32, kind="ExternalOutput").ap()

    with tile.TileContext(nc) as tc:
        _emit(tc, xT, qkv_wT, out_wT, ffn_w1T, ffn_w2T,
              qkv_b, out_b, ffn_b1, ffn_b2, maskT, yT)
    nc.compile()
    return nc


def _emit(tc, xT, qkv_wT, out_wT, ffn_w1T, ffn_w2T,
          qkv_b, out_b, ffn_b1, ffn_b2, maskT, yT):
    from contextlib import ExitStack
    ctx = ExitStack()
    nc = tc.nc

    sing = ctx.enter_context(tc.tile_pool(name="sing", bufs=1))
    wq_pool = ctx.enter_context(tc.tile_pool(name="wq", bufs=3))
    wv_pool = ctx.enter_context(tc.tile_pool(name="wv", bufs=1))
    w1_pool = ctx.enter_context(tc.tile_pool(name="w1", bufs=3))
    sq_pool = ctx.enter_context(tc.tile_pool(name="sq", bufs=2))
    ex_pool = ctx.enter_context(tc.tile_pool(name="ex", bufs=3))
    row_pool = ctx.enter_context(tc.tile_pool(name="rows", bufs=3))
    ow_pool = ctx.enter_context(tc.tile_pool(name="owp", bufs=2))
    rr_pool = ctx.enter_context(tc.tile_pool(name="rr", bufs=2))
    rf_pool = ctx.enter_context(tc.tile_pool(name="rf", bufs=1))
    den_pool = ctx.enter_context(tc.tile_pool(name="den", bufs=3))
    ft_pool = ctx.enter_context(tc.tile_pool(name="ftmp", bufs=2))

    ps_main = ctx.enter_context(tc.tile_pool(name="psg", bufs=2, space="PSUM"))
    ps_attn = ctx.enter_context(tc.tile_pool(name="pssc", bufs=3, space="PSUM"))
    ps_pv = ctx.enter_context(tc.tile_pool(name="pspv", bufs=3, space="PSUM"))
    ps_bc = ps_main

    # ---------------- phase 0: input DMAs ----------------
    x_sb = sing.tile([P, ET, SLAB], F32, tag="x_sb")
    xT_v = xT.rearrange("(o p) t -> p o t", p=P)
    for et in range(ET):
        nc.sync.dma_start(out=x_sb[:, et, :], in_=xT_v[:, et, :])

    qkvb_sb = sing.tile([P, 18], F32, tag="qkvb")
    nc.sync.dma_start(out=qkvb_sb, in_=qkv_b.rearrange("(o p) -> p o", p=P))
    outb_sb = sing.tile([P, ET], F32, tag="outb")
    nc.sync.dma_start(out=outb_sb, in_=out_b.rearrange("(o p) -> p o", p=P))
    b1_sb = sing.tile([P, MT], F32, tag="b1")
    nc.sync.dma_start(out=b1_sb, in_=ffn_b1.rearrange("(o p) -> p o", p=P))
    b2_sb = sing.tile([P, ET], F32, tag="b2")
    nc.sync.dma_start(out=b2_sb, in_=ffn_b2.rearrange("(o p) -> p o", p=P))

    # masks replicated over the head-pair dim: [key, qb, h2, kb, q]
    masks_sb = sing.tile([P, 2, 2, 2, P], BF16, tag="masks")
    for qb in range(2):
        for hrep in range(2):
            for kb in range(2):
                nc.sync.dma_start(out=masks_sb[:, qb, hrep, kb, :],
                                  in_=maskT[qb, kb])

    ones_pf = sing.tile([P, 1], BF16, tag="ones_pf")
    nc.vector.memset(ones_pf, 1.0)
    ones_row = sing.tile([1, P], BF16, tag="ones_row")
    nc.vector.memset(ones_row, 1.0)
    ones_row_f = sing.tile([1, P], F32, tag="ones_row_f")
    nc.vector.memset(ones_row_f, 1.0)
    eps_sb = sing.tile([1, 1], F32, tag="eps")
    nc.vector.memset(eps_sb, EPS)

    # dummy matmuls HAM-warm the PE clock while the input DMAs stream
    warm_src = sing.tile([P, 256], BF16, tag="warm_src")
    nc.gpsimd.memset(warm_src, 0.0)
    const_bf = nc.const_aps.aps[(mybir.dt.bfloat16, 1.0)]

    def emit_warmup(n, name):
        wps = ps_attn.tile([P, 2, 2, P], F32, tag="sc", name=name)
        for wi in range(n):
            nc.tensor.matmul(wps.rearrange("p a b c -> p (a b c)")[0:1, 0:256],
                             const_bf, warm_src, start=True, stop=True)

    emit_warmup(26, "warm_ps0")

    # ---------------- folded layernorm statistics ----------------
    def emit_ln_stats(src, xbf_dst, ntok, mu_row, arep_sb):
        """Compute LN statistics of src over the E axis.

        Writes: xbf_dst [P,ET,ntok] bf16 (plain cast of src, the GEMM input),
        mu_row [1,ntok] bf16 (mean), arep_sb [P,ntok] bf16 (rstd broadcast to
        all partitions).  The (x-mu)*rstd normalization itself is folded into
        the consuming GEMMs via mu_row and arep_sb."""
        chunks = [(0, 512), (512, ntok - 512)] if ntok > 512 else \
                 [(0, 256), (256, 256)]
        sts = [ps_main.tile([P, 512], F32, tag="g", name=f"st{ci}")
               for ci in range(len(chunks))]
        for et in range(ET):
            nc.vector.tensor_copy(out=xbf_dst[:, et, :ntok],
                                  in_=src[:, et, :ntok])
            xsq = sq_pool.tile([P, ntok], BF16, tag="sq")
            nc.scalar.activation(xsq, src[:, et, :ntok], AF.Square)
            for ci, (c0, cl) in enumerate(chunks):
                nc.tensor.matmul(sts[ci][0:1, :cl], ones_pf,
                                 xbf_dst[:, et, c0:c0 + cl],
                                 start=(et == 0), stop=(et == ET - 1))
                nc.tensor.matmul(sts[ci][32:33, :cl], ones_pf,
                                 xsq[:, c0:c0 + cl],
                                 start=(et == 0), stop=(et == ET - 1))
        for ci, (c0, cl) in enumerate(chunks):
            st = sts[ci]
            nc.scalar.activation(mu_row[:, c0:c0 + cl], st[0:1, :cl], AF.Copy,
                                 scale=1.0 / E)
            musq = row_pool.tile([1, 512], F32, tag="row")
            nc.scalar.activation(musq[:, :cl], st[0:1, :cl], AF.Square,
                                 scale=1.0 / E)
            var = row_pool.tile([1, 512], F32, tag="row")
            nc.vector.scalar_tensor_tensor(
                out=var[:, :cl], in0=st[32:33, :cl], scalar=1.0 / E,
                in1=musq[:, :cl], op0=OP.mult, op1=OP.subtract)
            std = row_pool.tile([1, 512], F32, tag="row")
            nc.scalar.activation(std[:, :cl], var[:, :cl], AF.Sqrt, bias=eps_sb)
            af = row_pool.tile([1, 512], F32, tag="row")
            nc.vector.reciprocal_approx_fast(out=af[:, :cl], in_=std[:, :cl])
            a = row_pool.tile([1, 512], BF16, tag="rowb")
            nc.vector.tensor_copy(out=a[:, :cl], in_=af[:, :cl])
            arep = ps_bc.tile([P, 512], F32, tag="g")
            nc.tensor.matmul(arep[:, :cl], ones_row, a[:, :cl],
                             start=True, stop=True)
            nc.vector.tensor_copy(out=arep_sb[:, c0:c0 + cl],
                                  in_=arep[:, :cl])
            murep = ps_bc.tile([P, 512], F32, tag="g")
            nc.tensor.matmul(murep[:, :cl], ones_row, mu_row[:, c0:c0 + cl],
                             start=True, stop=True)
            for et in range(ET):
                nc.vector.tensor_sub(xbf_dst[:, et, c0:c0 + cl],
                                     xbf_dst[:, et, c0:c0 + cl],
                                     murep[:, :cl])

    # ---------------- phase 1: LN1 stats ----------------
    x1_bf = sing.tile([P, ET, SLAB], BF16, tag="x1_bf")
    mu1 = sing.tile([1, SLAB], BF16, tag="mu1")
    a1rep = sing.tile([P, SLAB], BF16, tag="a1rep")
    emit_ln_stats(x_sb, x1_bf, SLAB, mu1, a1rep)
    emit_warmup(18, "warm_ps1")

    # rstd as per-partition columns for the V epilogue (tokens on partitions):
    # acol[:, p, kb] = rstd1 at the folded tokens of (parity p, block kb)
    acol_ps = ps_bc.tile([P, 512], F32, tag="g", name="acol_ps")
    one1 = ones_row[:, 0:1]
    for par in range(2):
        for kb in range(3):
            nc.tensor.matmul(acol_ps[:, par * 3 + kb:par * 3 + kb + 1],
                             _fold2(a1rep[0:1, :])[:, par, kb * P:(kb + 1) * P],
                             one1, start=(par == 0 and kb == 0),
                             stop=(par == 1 and kb == 2))
    acol_sb = sing.tile([P, 2, 3], F32, tag="acol")
    nc.vector.tensor_copy(out=acol_sb.rearrange("p a b -> p (a b)"),
                          in_=acol_ps[:, 0:6])

    # ---------------- phase 2: QKV projections (LN folded in) ----------------
    wq_view = qkv_wT.rearrange("(o p) f -> p o f", p=P)

    k_sb = sing.tile([P, ET, SLAB], BF16, tag="k_sb")
    for ft in range(ET):
        wt = wq_pool.tile([P, ET, P], BF16, tag="wq")
        nc.sync.dma_start(out=wt, in_=wq_view[:, :, E + ft * P:E + (ft + 1) * P])
        for c0, cl in [(0, 512), (512, 256)]:
            ps = ps_main.tile([P, 512], F32, tag="g")
            for et in range(ET):
                nc.tensor.matmul(ps[:, :cl], wt[:, et, :],
                                 x1_bf[:, et, c0:c0 + cl],
                                 start=(et == 0), stop=(et == ET - 1))
            t = ft_pool.tile([P, 512], BF16, tag="ftb")
            nc.vector.tensor_mul(t[:, :cl], ps[:, :cl], a1rep[:, c0:c0 + cl])
            nc.scalar.activation(k_sb[:, ft, c0:c0 + cl], t[:, :cl],
                                 AF.Identity, bias=qkvb_sb[:, 6 + ft:7 + ft])

    q_sb = sing.tile([P, ET, OWN], BF16, tag="q_sb")
    for ft in range(ET):
        wt = wq_pool.tile([P, ET, P], BF16, tag="wq")
        nc.sync.dma_start(out=wt, in_=wq_view[:, :, ft * P:(ft + 1) * P])
        ps = ps_main.tile([P, 512], F32, tag="g")
        for et in range(ET):
            nc.tensor.matmul(ps, wt[:, et, :], x1_bf[:, et, HALO:SLAB],
                             start=(et == 0), stop=(et == ET - 1))
        t = ft_pool.tile([P, 512], BF16, tag="ftb")
        nc.vector.tensor_mul(t, ps, a1rep[:, HALO:SLAB])
        nc.scalar.activation(q_sb[:, ft, :], t, AF.Identity,
                             bias=qkvb_sb[:, ft:ft + 1])

    # V in [token, feature] orientation; LN fold: the rank-1 mu term uses the
    # folded-token mu as lhsT, and rstd applies per-partition via ACT scale.
    # The V bias is folded into out_b on the host.
    v_sb = sing.tile([P, 2, 3, H, D + 1], BF16, tag="v_sb")
    nc.vector.memset(v_sb[:, :, :, :, D:D + 1], 1.0)
    for vc0, vcl in [(0, 512), (512, 256)]:
        wt = wv_pool.tile([P, ET, vcl], BF16, tag=f"wv{vcl}", name=f"wtv{vcl}")
        nc.sync.dma_start(out=wt,
                          in_=wq_view[:, :, 2 * E + vc0:2 * E + vc0 + vcl])
        for kb in range(3):
            for par in range(2):
                ps = ps_main.tile([P, 512], F32, tag="g")
                for et in range(ET):
                    hblk = _fold2(x1_bf[:, et, :])[:, par, kb * P:(kb + 1) * P]
                    nc.tensor.matmul(ps[:, :vcl], hblk, wt[:, et, :vcl],
                                     start=(et == 0), stop=(et == ET - 1))
                nc.scalar.activation(
                    v_sb[:, par, kb, vc0 // D:(vc0 + vcl) // D, 0:D],
                    ps[:, :vcl].rearrange("p (h d) -> p h d", d=D), AF.Copy,
                    scale=acol_sb[:, par, kb:kb + 1])

    outw_v = out_wT.rearrange("(o p) e -> p o e", p=P)

    # ---------------- phase 3: dilated attention ----------------
    o_sb = sing.tile([P, ET, OWN], BF16, tag="o_sb")
    for h0, h1 in [(0, 2), (1, 3), (4, 6), (5, 7), (8, 10), (9, 11)]:
        kt = h0 // 2
        ro = D * (h0 % 2)
        r2 = rr_pool.tile([1, 2, OWN], BF16, tag="r2",
                          name=f"r2_{h0}_{h1}")
        r2f = rf_pool.tile([1, 2, OWN], F32, tag="r2f",
                           name=f"r2f_{h0}_{h1}")
        for par in range(2):
            for qb in range(2):
                sc = ps_attn.tile([P, 2, 2, P], F32, tag="sc")
                for hi, h in enumerate((h0, h1)):
                    ktt = h // 2
                    qv = _fold2(q_sb[ro:ro + D, ktt, :])[:, par,
                                                         qb * P:(qb + 1) * P]
                    kv = _fold2(k_sb[ro:ro + D, ktt, :])
                    for kbi, kb in enumerate((qb, qb + 1)):
                        nc.tensor.matmul(
                            sc[:, hi, kbi, :],
                            kv[:, par, kb * P:(kb + 1) * P], qv,
                            start=(hi == 0 and kbi == 0),
                            stop=(hi == 1 and kbi == 1))
                ex = ex_pool.tile([P, 2, 2, P], BF16, tag="ex")
                nc.scalar.activation(ex, sc, AF.Exp, scale=1.0 / np.sqrt(D))
                nc.gpsimd.tensor_mul(ex, ex, masks_sb[:, qb])
                pv = ps_pv.tile([D + 1, 2, P], F32, tag="pv")
                for hi, h in enumerate((h0, h1)):
                    for kbi, kb in enumerate((qb, qb + 1)):
                        nc.tensor.matmul(
                            pv[:, hi, :], v_sb[:, par, kb, h, :],
                            ex[:, hi, kbi, :],
                            start=(hi == 0 and kbi == 0),
                            stop=(hi == 1 and kbi == 1))
                den = den_pool.tile([1, 2, P], F32, tag="den")
                nc.vector.tensor_copy(out=den, in_=pv[D:D + 1, :, :])
                nc.vector.reciprocal_approx_fast(
                    out=_fold2(r2f)[:, :, par, qb * P:(qb + 1) * P],
                    in_=den)
                dst = _fold2(o_sb[ro:ro + D, kt:kt + 2, :])[:, :, par,
                                                            qb * P:(qb + 1) * P]
                nc.vector.tensor_copy(out=dst, in_=pv[0:D])
        with nc.allow_low_precision(reason="bf16 softmax denom"):
            nc.vector.tensor_copy(out=r2, in_=r2f)
        # batched per-pair normalization
        for hi, tt in enumerate((kt, kt + 1)):
            rrep = ps_bc.tile([P, 512], F32, tag="g")
            nc.tensor.matmul(rrep[0:D, :], ones_row[:, 0:D], r2[:, hi, :],
                             start=True, stop=True)
            nc.vector.tensor_mul(o_sb[ro:ro + D, tt, :],
                                 o_sb[ro:ro + D, tt, :], rrep[0:D, :])

    # ---------------- phase 4: out-proj + residual ----------------
    y1_sb = sing.tile([P, ET, OWN], F32, tag="y1_sb")
    for et in range(ET):
        owt = ow_pool.tile([P, ET, P], BF16, tag="ow")
        nc.sync.dma_start(out=owt, in_=outw_v[:, :, et * P:(et + 1) * P])
        ps = ps_main.tile([P, 512], F32, tag="g")
        for ftl in range(ET):
            nc.tensor.matmul(ps, owt[:, ftl, :],
                             o_sb[:, ftl, :],
                             start=(ftl == 0), stop=(ftl == ET - 1))
        t = ft_pool.tile([P, 512], F32, tag="ft")
        nc.scalar.activation(t, ps, AF.Identity, bias=outb_sb[:, et:et + 1])
        nc.vector.tensor_add(y1_sb[:, et, :], t, x_sb[:, et, HALO:SLAB])

    # ---------------- phase 5: LN2 stats ----------------
    y1_bf = sing.tile([P, ET, OWN], BF16, tag="y1_bf")
    mu2 = sing.tile([1, OWN], BF16, tag="mu2")
    a2rep = sing.tile([P, OWN], BF16, tag="a2rep")
    emit_ln_stats(y1_sb, y1_bf, OWN, mu2, a2rep)

    # ---------------- phase 6: FFN1 + GELU (LN folded in) ----------------
    w2_sb = sing.tile([P, MT, E], BF16, tag="w2")
    w2_v = ffn_w2T.rearrange("(o p) e -> p o e", p=P)
    for ktl in range(MT):
        nc.sync.dma_start(out=w2_sb[:, ktl, :], in_=w2_v[:, ktl, :])

    w1_view = ffn_w1T.rearrange("(o p) f -> p o f", p=P)
    ffnh = sing.tile([P, MT, OWN], BF16, tag="ffnh")
    for mt in range(MT):
        wt = w1_pool.tile([P, ET, P], BF16, tag="w1")
        nc.sync.dma_start(out=wt, in_=w1_view[:, :, mt * P:(mt + 1) * P])
        ps = ps_main.tile([P, 512], F32, tag="g")
        for et in range(ET):
            nc.tensor.matmul(ps, wt[:, et, :], y1_bf[:, et, :],
                             start=(et == 0), stop=(et == ET - 1))
        t = ft_pool.tile([P, 512], BF16, tag="ftb")
        nc.vector.tensor_mul(t, ps, a2rep)
        nc.scalar.activation(ffnh[:, mt, :], t, AF.Gelu,
                             bias=b1_sb[:, mt:mt + 1])

    # ---------------- phase 7: FFN2 + residual + store ----------------
    yT_view = yT.rearrange("(o p) t -> p o t", p=P)
    for et in range(ET):
        ps = ps_main.tile([P, 512], F32, tag="g")
        for ktl in range(MT):
            nc.tensor.matmul(ps, w2_sb[:, ktl, et * P:(et + 1) * P],
                             ffnh[:, ktl, :],
                             start=(ktl == 0), stop=(ktl == MT - 1))
        t = ft_pool.tile([P, 512], F32, tag="ft")
        nc.scalar.activation(t, ps, AF.Identity, bias=b2_sb[:, et:et + 1])
        nc.vector.tensor_add(y1_sb[:, et, :], t, y1_sb[:, et, :])
        nc.sync.dma_start(out=yT_view[:, et, :], in_=y1_sb[:, et, :])

    ctx.close()


# ======================= host side =======================

def prep_inputs(x, ln1_w, ln1_b, qkv_w, qkv_b, out_w, out_b,
                ln2_w, ln2_b, ffn_w1, ffn_b1, ffn_w2, ffn_b2):
    """Shard/fold/cast the full inputs into 8 per-core input maps."""
    x = np.asarray(x, np.float32)
    f8 = lambda v: np.asarray(v, np.float64)

    qkv_wp = f8(qkv_w) * f8(ln1_w)[None, :]
    qkv_wT = qkv_wp.T.astype(BF16NP).copy()
    qkv_b_eff = (f8(qkv_b) + f8(qkv_w) @ f8(ln1_b)).astype(np.float32)
    out_wT = f8(out_w).T.astype(BF16NP).copy()
    out_b_eff = (f8(out_b) + f8(out_w) @ f8(qkv_b)[2 * E:]).astype(np.float32)
    ffn_w1p = f8(ffn_w1) * f8(ln2_w)[None, :]
    ffn_w1T = ffn_w1p.T.astype(BF16NP).copy()
    ffn_b1_eff = (f8(ffn_b1) + f8(ffn_w1) @ f8(ln2_b)).astype(np.float32)
    ffn_w2T = f8(ffn_w2).T.astype(BF16NP).copy()
    ffn_b2_f = np.asarray(ffn_b2, np.float32)

    cidx = np.arange(P)[:, None]   # key (folded, within block)
    ridx = np.arange(P)[None, :]   # query (folded, within block)
    m_prev = (cidx >= ridx).astype(BF16NP)
    m_diag = (cidx <= ridx).astype(BF16NP)
    zero = np.zeros((P, P), BF16NP)

    in_maps = []
    for c in range(N_CORES):
        b, ch = divmod(c, 4)
        lo = OWN * ch - HALO
        if ch == 0:
            slab = np.concatenate(
                [np.zeros((HALO, E), np.float32), x[b, 0:OWN]], axis=0)
        else:
            slab = x[b, lo:lo + SLAB]
        xTc = np.ascontiguousarray(slab.T)

        mask = np.stack([
            np.stack([zero if ch == 0 else m_prev, m_diag]),  # qb = 0
            np.stack([m_prev, m_diag]),                       # qb = 1
        ]).astype(BF16NP)

        in_maps.append({
            "xT": xTc,
            "qkv_wT": qkv_wT, "out_wT": out_wT,
            "ffn_w1T": ffn_w1T, "ffn_w2T": ffn_w2T,
            "qkv_b": qkv_b_eff, "out_b": out_b_eff,
            "ffn_b1": ffn_b1_eff, "ffn_b2": ffn_b2_f,
            "maskT": np.ascontiguousarray(mask),
        })
    return in_maps


def gather_output(results):
    y = np.empty((B, L, E), np.float32)
    for c in range(N_CORES):
        b, ch = divmod(c, 4)
        y[b, OWN * ch:OWN * (ch + 1)] = results[c]["yT"].T
    return y


_NC_CACHE = None


def _get_program():
    global _NC_CACHE
    if _NC_CACHE is None:
        _NC_CACHE = build_program()
    return _NC_CACHE


def kernel(**inputs):
    nc = _get_program()
    in_maps = prep_inputs(**inputs)
    res = run_bass_kernel_spmd(nc, in_maps, core_ids=list(range(N_CORES)))
    return gather_output(res.results)

